# revision 19
# baseline (speedup 1.0000x reference)
"""Trainium2 Bass kernel for nn_Attention_54254026883778.

Single-head attention with an additive post-softmax intensity term:
    q/k/v = X @ W{q,k,v}.T + b;  scores = q k^T / sqrt(D)
    attn  = softmax(scores) + intensity;  out = (attn @ v) @ Wo.T + bo

Sharding: 8 cores = 4 batches x 2 sequence halves. Each core computes
Q^T for its own 1024 query rows, the FULL K^T for its batch (redundant
fp8 compute is cheaper and far more robust than exchanging K halves:
scores then depends on no collective at all), and V' for its own rows;
only the V halves are exchanged, via two staged 2-rank AllGathers whose
results are not needed until the PV phase ~40us later -- so collective
latency and cross-core launch skew are fully absorbed.

Math restructuring (host side, exact):
    Wvo = Wo @ Wv  =>  out = attn @ (X @ Wvo.T) + rowsum(attn) (x) (Wo@bv) + bo
which deletes the output projection GEMM. rowsum(attn) = 1 + rowsum(I)
is shipped from the host.

Precision: all large GEMMs except V' run in fp8 e4m3 with
MatmulPerfMode.DoubleRow (256-row contraction per instruction, 2x bf16
throughput):
  - Q/K projections + scores: weights pre-scaled by 32 on the host so
    their uniform(+-1/32) entries stay in the e4m3 normal range; the
    32*32 factor is folded into the softmax exp scale.
  - PV runs fp8 on DOUBLE-CENTERED operands: attn_c = attn - 0.5 (the
    host ships intensity^T - 0.5) and V_c = V' - m, where m is the
    column mean of V'. Centering routes the large attn/V' means through
    the exact f32 bias tensor:
       BIAS[do,s] = (Wo@bv + m)[do] * rowsums[s] + bo[do]
  - V'-proj itself stays bf16 (X, Wvo bf16): V'-path input errors are
    amplified by ~sqrt(S)*|attn| in attn @ V', so fp8 inputs there
    blow the error budget (measured: 1.47e-2 -> 1.85e-2 rel).

Schedule notes:
  - All GEMM inputs load into PER-PAIR / PER-CHUNK SBUF tiles from
    pair-contiguous host layouts: every DMA moves 2KB+ per partition and
    each matmul waits only on its own pair, so the PE starts ~4-5us into
    the DMA ramp and the ramp itself warms the PE_HAM clock gate.
  - The V staging DMAs ride the scalar-family queues (short queue), the
    IT prefetch is emitted just before the scores loop and BIAS after it,
    keeping the early sync queues lean so the V collectives' mesh DMA is
    not starved behind header bulk.

Device dataflow (all t in global order; per-core tensors from host):
    K^T  [dout | t full] = WK8-chunk.T @ XF8   (fp8 DR, full batch rows)
    V'   [t-own | dv] = XH16-chunk.T @ WVO16   (bf16, own half) - m -> fp8
        -> two staged AllGathers (t-chunks 0..3, then 4..7) -> V full
    Q^T  [dout | s]   = WQ8-chunk.T @ XQ8      (fp8 DR, own rows)
    scores^T [t | s]  = KT-chunk.T @ QT        (fp8 DR, directly in the
        transposed orientation PV needs -- no PE transposes) -> exp on
        ACT (scale 1/32768, no max-subtract: |scores|<~3) -> fp8 E^T
        den[s] = ones.T @ E^T (8 accumulating DR matmuls, 1-partition
        PSUM row) -> 1/den on DVE -> broadcast to 128 partitions via a
        1-contraction PE matmul -> attn_c^T = E^T*recip + (I^T-0.5)
        (two 16-bit DVE passes) -> fp8
    out^T [do | s]    = V-chunk.T @ attn_c^T   (fp8 DR) -> DVE adds the
        host BIAS tile while draining PSUM -> DRAM (f32), host transposes.
"""

import numpy as np
import ml_dtypes

P = 128
D = 1024
S = 2048          # keys per batch (full sequence)
SH = 1024         # query rows owned by each core
DC = D // P       # 8  contraction chunks over model dim
DC2 = DC // 2     # 4  DoubleRow chunk-pairs
TC = S // P       # 16 t (key) chunks
TC2 = TC // 2     # 8  DoubleRow t-chunk pairs
NT = 512          # matmul moving free dim / psum bank
SJ = SH // NT     # 2  s-tiles of own rows
TJ = S // NT      # 4  t-tiles
WS = 32.0         # host pre-scale on Wq/Wk (keeps fp8 in normal range)
SCALE = 1.0 / (32.0 * WS * WS)  # 1/sqrt(D) / WS^2

_CACHE = {}


def _build_module():
    import concourse.bass as bass
    import concourse.tile as tile
    import concourse.mybir as mybir
    from concourse import bacc

    f32 = mybir.dt.float32
    bf16 = mybir.dt.bfloat16
    fp8 = mybir.dt.float8e4
    DR = mybir.MatmulPerfMode.DoubleRow
    Exp = mybir.ActivationFunctionType.Exp
    Copy = mybir.ActivationFunctionType.Copy
    add = mybir.AluOpType.add
    sub = mybir.AluOpType.subtract
    mult = mybir.AluOpType.mult

    nc = bacc.Bacc("TRN2", target_bir_lowering=False, debug=False,
                   num_devices=8)

    # pair-contiguous host layouts: one DMA per DoubleRow chunk-pair,
    # 2KB+ per partition per transfer.
    XF_d = nc.dram_tensor("XF8", [DC2, P, 2, S], fp8, kind="ExternalInput")
    XQ_d = nc.dram_tensor("XQ8", [DC2, P, 2, SH], fp8, kind="ExternalInput")
    WQ_d = nc.dram_tensor("WQ8", [DC2, P, 2, D], fp8, kind="ExternalInput")
    WK_d = nc.dram_tensor("WK8", [DC2, P, 2, D], fp8, kind="ExternalInput")
    XH_d = nc.dram_tensor("XH16", [DC, P, SH], bf16, kind="ExternalInput")
    WVO_d = nc.dram_tensor("WVO16", [DC, P, D], bf16, kind="ExternalInput")
    M_d = nc.dram_tensor("M16", [P, D], bf16, kind="ExternalInput")
    BCOL_d = nc.dram_tensor("BCOL", [P, 2 * DC], f32, kind="ExternalInput")
    BIAS_d = nc.dram_tensor("BIAS", [D, SH], f32, kind="ExternalInput")
    IT_d = nc.dram_tensor("IT16", [SJ * TC * P, NT], bf16,
                          kind="ExternalInput")
    OUT_d = nc.dram_tensor("OUTT", [D, SH], f32, kind="ExternalOutput")

    # IT^T [st, tb, tp, s-in-tile]: one contiguous [P, NT] block per
    # (s-tile, t-chunk) pair
    it_v = IT_d[:].rearrange("(st tb p) f -> st tb p f", tb=TC, p=P)
    bias_v = BIAS_d[:].rearrange("(c p) s -> p c s", p=P)
    out_v = OUT_d[:].rearrange("(c p) s -> p c s", p=P)

    GROUPS = [[0, 1], [2, 3], [4, 5], [6, 7]]

    with tile.TileContext(nc) as tc:
        with (
            tc.tile_pool(name="persist", bufs=1) as persist,
            tc.tile_pool(name="mm_ps", bufs=5, space="PSUM") as mm_ps,
            tc.tile_pool(name="tr_ps", bufs=1, space="PSUM") as tr_ps,
            tc.tile_pool(name="dram", bufs=1, space="DRAM") as dram_pool,
            tc.tile_pool(name="e_pool", bufs=2) as e_pool,
            # shared 256KB-tile ring: XH/WVO chunks live through the V'
            # phase, then the same buffers recycle as IT^T tiles during
            # the scores phase (16 concurrently live per s-tile)
            tc.tile_pool(name="bf256", bufs=18) as bf256,
            tc.tile_pool(name="stat_pool", bufs=2) as stat_pool,
            tc.tile_pool(name="bias_pool", bufs=4) as bias_pool,
            tc.tile_pool(name="fin_pool", bufs=3) as fin_pool,
        ):
            # ---- persistent tiles (per-pair for granular DMA waits) ---
            WKp = [persist.tile([P, 2, D], fp8, name=f"wk{i}")
                   for i in range(DC2)]
            WQp = [persist.tile([P, 2, D], fp8, name=f"wq{i}")
                   for i in range(DC2)]
            XFp = [persist.tile([P, 2, S], fp8, name=f"xf{i}")
                   for i in range(DC2)]
            XQp = [persist.tile([P, 2, SH], fp8, name=f"xq{i}")
                   for i in range(DC2)]
            XHc = [bf256.tile([P, SH], bf16, tag="it", name=f"xh{i}")
                   for i in range(DC)]
            WVOc = [bf256.tile([P, D], bf16, tag="it", name=f"wvo{i}")
                    for i in range(DC)]
            KL_sb = persist.tile([P, DC, S], fp8)      # K^T [dout | t full]
            QT_sb = persist.tile([P, DC, SH], fp8)     # Q^T [dout | s]
            VL_sb = persist.tile([P, DC, SH], fp8)     # own V_c [t | dv]
            V_sb = persist.tile([P, TC, D], fp8)       # full V_c [t | dv]
            AT_sb = persist.tile([P, TC, SH], fp8)     # attn_c^T [t | s]
            M_sb = persist.tile([P, D], bf16)          # colmean(V') rows
            BCOL_sb = persist.tile([P, 2 * DC], f32)
            ones2 = persist.tile([P, 2, 16], fp8)      # DR ones stationary
            onesr = persist.tile([1, P], f32)          # bcast stationary
            nc.vector.memset(ones2[:], 1.0)
            nc.vector.memset(onesr[:], 1.0)

            # Critical first payload on the scalar family (its queues
            # come out of init first); bulk on sync.
            nc.scalar.dma_start(WKp[0][:], WK_d[0])
            for i in range(DC2):
                nc.scalar.dma_start(XFp[i][:], XF_d[i])
            nc.sync.dma_start(BCOL_sb[:], BCOL_d[:])
            for i in range(1, DC2):
                nc.sync.dma_start(WKp[i][:], WK_d[i])
            nc.sync.dma_start(M_sb[:], M_d[:])
            for i in range(DC):
                nc.sync.dma_start(XHc[i][:], XH_d[i])
                nc.sync.dma_start(WVOc[i][:], WVO_d[i])
            for i in range(DC2):
                nc.sync.dma_start(WQp[i][:], WQ_d[i])
                nc.sync.dma_start(XQp[i][:], XQ_d[i])

            # ---- K^T (fp8 DoubleRow), FULL batch rows (no exchange) ---
            for c in range(DC):
                psl = [mm_ps.tile([P, NT], f32, tag="mm", name="ps")
                       for _ in range(TJ)]
                for dc2 in range(DC2):
                    for tj in range(TJ):
                        nc.tensor.matmul(
                            psl[tj][:],
                            WKp[dc2][:, :, c * P:(c + 1) * P],
                            XFp[dc2][:, :, tj * NT:(tj + 1) * NT],
                            start=(dc2 == 0), stop=(dc2 == DC2 - 1),
                            perf_mode=DR,
                        )
                for tj in range(TJ):
                    nc.vector.tensor_scalar_add(
                        KL_sb[:, c, tj * NT:(tj + 1) * NT], psl[tj][:],
                        BCOL_sb[:, DC + c:DC + c + 1])

            # ---- V' (bf16) own half, centered -> fp8; staged exchange -
            # Before Q-proj: V-half-1's gather latency hides under Q and
            # the whole scores phase.
            for half in range(2):
                for t in range(half * DC // 2, (half + 1) * DC // 2):
                    psl = [mm_ps.tile([P, NT], f32, tag="mm", name="ps")
                           for _ in range(D // NT)]
                    for dc in range(DC):
                        for j in range(D // NT):
                            nc.tensor.matmul(
                                psl[j][:],
                                XHc[dc][:, t * P:(t + 1) * P],
                                WVOc[dc][:, j * NT:(j + 1) * NT],
                                start=(dc == 0), stop=(dc == DC - 1),
                            )
                    for j in range(D // NT):
                        nc.vector.tensor_tensor(
                            VL_sb[:, t, j * NT:(j + 1) * NT],
                            psl[j][:], M_sb[:, j * NT:(j + 1) * NT], sub)
                HC = DC // 2
                v_in = dram_pool.tile([P, HC, SH], fp8, name=f"v_in{half}")
                v_out = dram_pool.tile([2, P, HC, SH], fp8,
                                       name=f"v_out{half}")
                # staging rides the (short) scalar queues so the gather
                # isn't gated on the sync-family header bulk
                nc.scalar.dma_start(v_in[:],
                                    VL_sb[:, half * HC:(half + 1) * HC])
                nc.gpsimd.collective_compute(
                    "AllGather", mybir.AluOpType.bypass,
                    replica_groups=GROUPS,
                    ins=[v_in.opt()], outs=[v_out.opt()])
                # rank order == global t order: rank 0 rows then rank 1
                nc.sync.dma_start(V_sb[:, half * HC:(half + 1) * HC], v_out[0])
                nc.sync.dma_start(V_sb[:, DC + half * HC:DC + (half + 1) * HC],
                                  v_out[1])

            # ---- Q^T (fp8 DoubleRow), own rows ------------------------
            for c in range(DC):
                psl = [mm_ps.tile([P, NT], f32, tag="mm", name="ps")
                       for _ in range(SJ)]
                for dc2 in range(DC2):
                    for sj in range(SJ):
                        nc.tensor.matmul(
                            psl[sj][:],
                            WQp[dc2][:, :, c * P:(c + 1) * P],
                            XQp[dc2][:, :, sj * NT:(sj + 1) * NT],
                            start=(dc2 == 0), stop=(dc2 == DC2 - 1),
                            perf_mode=DR,
                        )
                for sj in range(SJ):
                    nc.vector.tensor_scalar_add(
                        QT_sb[:, c, sj * NT:(sj + 1) * NT], psl[sj][:],
                        BCOL_sb[:, c:c + 1])

            # ---- scores^T -> softmax -> +(I^T-0.5) -> attn_c^T (fp8) --
            # Scores are computed directly TRANSPOSED (stationary = K^T
            # chunk, moving = Q^T), so no per-tile PE transposes or diag
            # chain are needed. Per s-tile of 512 queries:
            #   E^T[t, s] = exp(scale * scores^T)            (ACT -> fp8)
            #   den[s]    = ones.T @ E^T  (8 accumulating DR matmuls into
            #               a 1-partition PSUM row)
            #   recip     = 1/den on DVE; broadcast to 128 partitions by
            #               a 1-contraction PE matmul; ACT-copied to bf16
            #   attn_c^T  = E^T * recip + IT^T               (2 DVE passes)
            for st in range(SJ):
                E8 = e_pool.tile([P, TC, NT], fp8, tag="e")
                IT_t = []
                for tb in range(TC):
                    itt = bf256.tile([P, NT], bf16, tag="it")
                    nc.sync.dma_start(itt[:], it_v[st, tb])
                    IT_t.append(itt)
                    ps = mm_ps.tile([P, NT], f32, tag="mm", name="ps")
                    for dc2 in range(DC2):
                        nc.tensor.matmul(
                            ps[:],
                            KL_sb[:, 2 * dc2:2 * dc2 + 2, tb * P:(tb + 1) * P],
                            QT_sb[:, 2 * dc2:2 * dc2 + 2,
                                  st * NT:(st + 1) * NT],
                            start=(dc2 == 0), stop=(dc2 == DC2 - 1),
                            perf_mode=DR,
                        )
                    nc.scalar.activation(E8[:, tb, :], ps[:], Exp,
                                         scale=SCALE)
                dps = tr_ps.tile([1, NT], f32, tag="den")
                for tc2 in range(TC2):
                    nc.tensor.matmul(
                        dps[:], ones2[:, :, 0:1], E8[:, 2 * tc2:2 * tc2 + 2, :],
                        start=(tc2 == 0), stop=(tc2 == TC2 - 1),
                        perf_mode=DR,
                    )
                recip = stat_pool.tile([1, NT], f32, tag="recip")
                nc.vector.reciprocal(recip[:], dps[:])
                rps = tr_ps.tile([P, NT], f32, tag="bcast")
                nc.tensor.matmul(rps[:], onesr[:], recip[:],
                                 start=True, stop=True)
                Rb = stat_pool.tile([P, NT], bf16, tag="rb")
                nc.scalar.activation(Rb[:], rps[:], Copy)
                for tb in range(TC):
                    T1 = fin_pool.tile([P, NT], bf16, tag="t1")
                    nc.vector.tensor_tensor(T1[:], E8[:, tb, :], Rb[:], mult)
                    nc.vector.tensor_tensor(
                        AT_sb[:, tb, st * NT:(st + 1) * NT],
                        T1[:], IT_t[tb][:], add)

            # ---- PV (fp8 DoubleRow): out^T = V_c.T @ attn_c^T + BIAS --
            # sj-major: the sj=0 pass reads only attn columns from the
            # st=0 s-tile, so it can start while st=1's DVE drains run.
            # BIAS halves stream through a small pool (loads emitted
            # here, after everything the scores phase depends on).
            def emit_pv(sj, last):
                for dvi in range(DC):
                    B_sb = bias_pool.tile([P, NT], f32, tag="bias")
                    nc.sync.dma_start(B_sb[:],
                                      bias_v[:, dvi, sj * NT:(sj + 1) * NT])
                    ps = mm_ps.tile([P, NT], f32, tag="mm", name="ps")
                    for tc2 in range(TC2):
                        nc.tensor.matmul(
                            ps[:],
                            V_sb[:, 2 * tc2:2 * tc2 + 2, dvi * P:(dvi + 1) * P],
                            AT_sb[:, 2 * tc2:2 * tc2 + 2, sj * NT:(sj + 1) * NT],
                            start=(tc2 == 0), stop=(tc2 == TC2 - 1),
                            perf_mode=DR,
                        )
                    # the last group drains in quarter tiles so the final
                    # extract+store after the last matmul is short
                    NQ = NT // 4 if (last and dvi == DC - 1) else NT
                    for q in range(NT // NQ):
                        lo = sj * NT + q * NQ
                        F_sb = fin_pool.tile([P, NQ], f32, tag="fin",
                                             name=f"fin{sj}_{dvi}_{q}")
                        nc.vector.tensor_tensor(
                            F_sb[:], ps[:, q * NQ:(q + 1) * NQ],
                            B_sb[:, q * NQ:(q + 1) * NQ], add)
                        nc.sync.dma_start(
                            out_v[:, dvi, lo:lo + NQ], F_sb[:])

            emit_pv(0, last=False)
            emit_pv(1, last=True)

    nc.compile()
    return nc


def _get_module():
    if "nc" not in _CACHE:
        _CACHE["nc"] = _build_module()
    return _CACHE["nc"]


def _pairs(a):
    """[D, N] -> [DC2, P, 2, N] pair-contiguous layout."""
    n = a.shape[1]
    return np.ascontiguousarray(
        a.reshape(DC2, 2, P, n).transpose(0, 2, 1, 3))


def _make_in_maps(inputs):
    X = np.asarray(inputs["X"], dtype=np.float32)
    intensity = np.asarray(inputs["intensity"], dtype=np.float32)
    bf = ml_dtypes.bfloat16
    f8 = ml_dtypes.float8_e4m3
    Wq = np.asarray(inputs["Wq"], np.float32)
    Wk = np.asarray(inputs["Wk"], np.float32)
    Wv = np.asarray(inputs["Wv"], np.float32)
    Wo = np.asarray(inputs["Wo"], np.float32)
    Wvo = (Wo.astype(np.float64) @ Wv.astype(np.float64))  # fused V/O proj
    WQ8 = _pairs((WS * Wq).T).astype(f8)
    WK8 = _pairs((WS * Wk).T).astype(f8)
    WVO16 = np.ascontiguousarray(
        Wvo.T.astype(np.float32)).reshape(DC, P, D).astype(bf)
    bq, bk, bv, bo = (np.asarray(inputs[k], np.float32).reshape(D)
                      for k in ("bq", "bk", "bv", "bo"))
    bvo = (Wo.astype(np.float64) @ bv.astype(np.float64))
    BCOL = np.concatenate(
        [(WS * b).reshape(DC, P).T for b in (bq, bk)], axis=1
    ).astype(np.float32)  # [128, 16]

    in_maps = []
    for c in range(8):
        b, h = c // 2, c % 2
        XTF = np.ascontiguousarray(X[b].T)                        # [D, S]
        XT = np.ascontiguousarray(XTF[:, h * SH:(h + 1) * SH])    # [D, SH]
        XF8 = _pairs(XTF).astype(f8)
        XQ8 = _pairs(XT).astype(f8)
        XH16 = XT.reshape(DC, P, SH).astype(bf)
        # column mean of V' = colsum(X) @ Wvo.T / S, quantized to the
        # same bf16 the device subtracts so bias and centering agree.
        m = (X[b].sum(axis=0, dtype=np.float64) @ Wvo.T) / S
        m16 = m.astype(np.float32).astype(bf)
        M16 = np.broadcast_to(m16, (P, D)).copy()
        Islc = intensity[b, h * SH:(h + 1) * SH, :]
        # I^T - 0.5 laid out as [st, tb, tp, s-in-tile]: one contiguous
        # [P, NT] block per (s-tile, t-chunk) pair
        IT16 = np.ascontiguousarray(
            (Islc.T - 0.5).reshape(TC, P, SJ, NT)
            .transpose(2, 0, 1, 3).reshape(SJ * TC * P, NT)
        ).astype(bf)
        rows = 1.0 + Islc.sum(axis=1, dtype=np.float64)
        BIAS = ((bvo + m16.astype(np.float64))[:, None] * rows[None, :]
                + bo.astype(np.float64)[:, None]).astype(np.float32)
        in_maps.append({
            "XF8": XF8, "XQ8": XQ8, "XH16": XH16,
            "WQ8": WQ8, "WK8": WK8, "WVO16": WVO16, "M16": M16,
            "BCOL": BCOL, "BIAS": BIAS, "IT16": IT16,
        })
    return in_maps


def _gather(results):
    out = np.empty((4, S, D), dtype=np.float32)
    for c in range(8):
        b, h = c // 2, c % 2
        out[b, h * SH:(h + 1) * SH, :] = results[c]["OUTT"].T
    return out


def kernel(**inputs):
    from concourse import bass_utils

    in_maps = _make_in_maps(inputs)
    nc = _get_module()
    res = bass_utils.run_bass_kernel_spmd(nc, in_maps, core_ids=list(range(8)))
    return _gather(res.results)
